# revision 1
# baseline (speedup 1.0000x reference)
"""DeepSeek-MLA prefill kernel for 8 Trainium2 NeuronCores (head-parallel).

Contract: kernel(**inputs) takes the FULL unsharded inputs from
setup_inputs() and returns the FULL [1, 2048, 4096] float32 output.

Sharding (hardcoded for B=1, S=2048, H=4096, NH=32):
  - LoRA down-projections: sequence-parallel (256 rows/core). The kv path
    runs first and its (normed, roped) activations are AllGathered early,
    so the kv up-projections overlap the q-side gather.
  - Up-projections + attention: head-parallel, 4 heads/core.
  - Output projection: per-512-query-group AllGather of attention outputs
    pipelined against the next attention group; each core computes a
    512-column slice of out @ wo.T; host concatenates.
All matmuls run in bf16 with fp32 PSUM accumulation. Softmax runs on
transposed scores (keys on partitions): ACT exp straight from PSUM into
bf16 probs; denominators accumulate on DVE and reduce via a single
ones-matmul per query group; normalization uses GpSimd
partition-broadcast. Causal masking skips fully-masked key-chunks and
trims masked query columns from every stage including the score matmuls.
"""

import numpy as np
import ml_dtypes

import concourse.bass as bass
import concourse.mybir as mybir
import concourse.tile as tile
from concourse import bacc
from concourse.bass_utils import run_bass_kernel_spmd
from concourse.masks import make_identity

BF16 = mybir.dt.bfloat16
F32 = mybir.dt.float32
AF = mybir.ActivationFunctionType

NCORE = 8
B, S, H = 1, 2048, 4096
NH = 32
DN, DR, DV = 128, 64, 128  # qk_nope, qk_rope, v dims
LQ, LKV = 1536, 512
EPS = 1e-6
HL = NH // NCORE          # heads per core = 4
SC = S // NCORE           # local seq rows = 256
KVD = LKV + DR            # 576 kv-path feature dim
NEG = -1e5

_compiled = {}


def _build(collectives=True):
    nc = bacc.Bacc("TRN2", target_bir_lowering=False, debug=False,
                   num_devices=NCORE)

    x_t = nc.declare_dram_parameter("x_t", [H, SC], BF16, isOutput=False)
    wq_at = nc.declare_dram_parameter("wq_at", [H, LQ], BF16, isOutput=False)
    wkv_at = nc.declare_dram_parameter("wkv_at", [H, KVD], BF16, isOutput=False)
    wqbn_t = nc.declare_dram_parameter("wqbn_t", [LQ, HL * DN], BF16, isOutput=False)
    wqbr_t = nc.declare_dram_parameter("wqbr_t", [LQ, HL * DR], BF16, isOutput=False)
    wkvbn_t = nc.declare_dram_parameter("wkvbn_t", [LKV, HL * DN], BF16, isOutput=False)
    wkvbv_t = nc.declare_dram_parameter("wkvbv_t", [LKV, HL * DV], BF16, isOutput=False)
    wo_t = nc.declare_dram_parameter("wo_t", [NH * DV, 512], BF16, isOutput=False)
    cos_k = nc.declare_dram_parameter("cos_k", [SC, DR // 2], F32, isOutput=False)
    sin_k = nc.declare_dram_parameter("sin_k", [SC, DR // 2], F32, isOutput=False)
    cos_r = nc.declare_dram_parameter("cos_r", [S, HL * DR // 2], BF16, isOutput=False)
    sin_r = nc.declare_dram_parameter("sin_r", [S, HL * DR // 2], BF16, isOutput=False)
    dmask = nc.declare_dram_parameter("dmask", [128, 128], F32, isOutput=False)
    out_c = nc.declare_dram_parameter("out_c", [S, 512], F32, isOutput=True)

    KQ = LQ // 128    # 12 lora k-chunks (q)
    KKV = LKV // 128  # 4
    KH = H // 128     # 32
    NT = S // 128     # 16 seq tiles
    NG = S // 512     # 4 seq groups
    NFKV = (KVD + 127) // 128   # 5 kv feature chunks (last is 64 wide)

    shared = "Shared" if collectives else "Local"

    with tile.TileContext(nc) as tc:
        with tc.tile_pool(name="consts", bufs=1) as consts, \
             tc.tile_pool(name="dram", bufs=1, space="DRAM") as dram:
            ident = consts.tile([128, 128], BF16)
            make_identity(nc, ident)
            ones_k = consts.tile([128, 1], BF16)
            nc.vector.memset(ones_k, 1.0)
            dmask_sb = consts.tile([128, 128], F32)
            nc.sync.dma_start(out=dmask_sb, in_=dmask[:, :])
            eps_sb = consts.tile([128, 1], F32)
            nc.vector.memset(eps_sb, EPS)

            ACT_D = LQ + KVD  # q rows 0:1536, kv rows 1536:2112
            bounce_act = dram.tile([ACT_D, SC], BF16)
            gath_act = dram.tile([NCORE, ACT_D, SC], BF16, addr_space=shared)
            bounce_o = dram.tile([HL * DV, S], BF16)
            gath_o = dram.tile([NCORE, HL * DV, S], BF16, addr_space=shared)

            def allgather(src, dst):
                if collectives:
                    nc.gpsimd.collective_compute(
                        "AllGather", mybir.AluOpType.bypass,
                        replica_groups=[list(range(NCORE))],
                        ins=[src.opt()], outs=[dst.opt()])
                else:
                    for r in range(NCORE):
                        nc.gpsimd.dma_start(out=dst[r], in_=src[:, :])

            # ---------------- Phase 1: LoRA down-proj on local rows ------
            from contextlib import ExitStack
            with tc.tile_pool(name="p1w", bufs=6) as p1w, \
                 tc.tile_pool(name="p1tps", bufs=2, space="PSUM") as p1tps, \
                 tc.tile_pool(name="p1sb", bufs=3) as p1sb, \
                 tc.tile_pool(name="p1acc", bufs=1) as p1acc:
                x_sb = []
                for i in range(2):
                    xt = p1sb.tile([128, KH // 2, SC], BF16, tag=f"xsb{i}",
                                   bufs=1, name=f"xsb{i}")
                    eng = nc.sync if i == 0 else nc.scalar
                    eng.dma_start(
                        out=xt,
                        in_=x_t[i * (H // 2):(i + 1) * (H // 2), :]
                            .rearrange("(kk p) s -> p kk s", p=128))
                    x_sb.append(xt)

                def xch(kk):
                    return x_sb[kk // 16][:, kk % 16, :]

                cosk_sb = p1sb.tile([128, 2, DR // 2], F32, tag="cosk", bufs=1)
                sink_sb = p1sb.tile([128, 2, DR // 2], F32, tag="sink", bufs=1)
                nc.sync.dma_start(out=cosk_sb, in_=cos_k.rearrange("(mt p) i -> p mt i", p=128))
                nc.sync.dma_start(out=sink_sb, in_=sin_k.rearrange("(mt p) i -> p mt i", p=128))

                # sweep 1: kv lora (stream wkv_at once)
                p1kv_stack = ExitStack()
                p1pskv = p1kv_stack.enter_context(
                    tc.tile_pool(name="p1pskv", bufs=1, space="PSUM"))
                pskv = [p1pskv.tile([128, 512], F32, tag=f"kv{mt}",
                                    name=f"pskv{mt}") for mt in range(2)]
                pskpe = [p1pskv.tile([128, DR], F32, tag=f"kvp{mt}",
                                     name=f"pskpe{mt}") for mt in range(2)]
                for kk in range(KH):
                    wv_sb = p1w.tile([128, KVD], BF16, tag="wv")
                    eng = nc.sync if kk % 2 == 0 else nc.scalar
                    eng.dma_start(out=wv_sb,
                                  in_=wkv_at[kk * 128:(kk + 1) * 128, :])
                    st, sp = kk == 0, kk == KH - 1
                    for mt in range(2):
                        lhs = xch(kk)[:, mt * 128:(mt + 1) * 128]
                        nc.tensor.matmul(pskv[mt], lhs, wv_sb[:, :512],
                                         start=st, stop=sp)
                        nc.tensor.matmul(pskpe[mt], lhs, wv_sb[:, 512:],
                                         start=st, stop=sp)

                kvact = []
                for mt in range(2):
                    actl = p1acc.tile([128, KVD], BF16, tag=f"kvact{mt}",
                                      name=f"kvact{mt}")
                    kvact.append(actl)
                    # rms norm kv (over 512)
                    sqv = p1sb.tile([128, 512], F32, tag="sqv")
                    ssv = p1sb.tile([128, 1], F32, tag="ssv")
                    nc.scalar.activation(sqv, pskv[mt], AF.Square,
                                         accum_out=ssv)
                    rtv = p1sb.tile([128, 1], F32, tag="rtv")
                    nc.scalar.activation(rtv, ssv, AF.Sqrt, bias=eps_sb,
                                         scale=1.0 / LKV)
                    rstdv = p1sb.tile([128, 1], F32, tag="rstdv")
                    nc.vector.reciprocal(rstdv, rtv)
                    nc.vector.tensor_scalar_mul(actl[:, :512], pskv[mt], rstdv)
                    # rope k_pe (no norm)
                    kv3 = pskpe[mt].rearrange("p (i two) -> p i two", two=2)
                    x1, x2 = kv3[:, :, 0], kv3[:, :, 1]
                    cs, sn = cosk_sb[:, mt, :], sink_sb[:, mt, :]
                    t1 = p1sb.tile([128, DR // 2], F32, tag="t1")
                    t2 = p1sb.tile([128, DR // 2], F32, tag="t2")
                    ko = actl[:, 512:].rearrange("p (i two) -> p i two", two=2)
                    nc.vector.tensor_mul(t1, x1, cs)
                    nc.vector.tensor_mul(t2, x2, sn)
                    nc.vector.tensor_sub(ko[:, :, 0], t1, t2)
                    nc.vector.tensor_mul(t1, x1, sn)
                    nc.vector.tensor_mul(t2, x2, cs)
                    nc.vector.tensor_add(ko[:, :, 1], t1, t2)

                p1kv_stack.close()

                # transpose kv activations -> [KVD, SC] and bounce+gather
                for mt in range(2):
                    for ft in range(NFKV):
                        fs = ft * 128
                        fw = min(128, KVD - fs)
                        tp = p1tps.tile([128, 128], BF16, tag="tp")
                        nc.tensor.transpose(tp[:fw, :], kvact[mt][:, fs:fs + fw],
                                            ident)
                        cp = p1sb.tile([128, 128], BF16, tag="cp")
                        nc.scalar.copy(cp[:fw, :], tp[:fw, :])
                        nc.sync.dma_start(
                            out=bounce_act[LQ + fs:LQ + fs + fw,
                                           mt * 128:(mt + 1) * 128],
                            in_=cp[:fw, :])

                # sweep 2: q lora (stream wq_at once, both mt tiles)
                p1q_stack = ExitStack()
                p1psq = p1q_stack.enter_context(
                    tc.tile_pool(name="p1psq", bufs=1, space="PSUM"))
                psq = [[p1psq.tile([128, 512], F32, tag=f"q{mt}{g}",
                                   name=f"psq{mt}{g}")
                        for g in range(3)] for mt in range(2)]
                for kk in range(KH):
                    w_sb = p1w.tile([128, LQ], BF16, tag="w")
                    eng = nc.sync if kk % 2 == 0 else nc.scalar
                    eng.dma_start(out=w_sb,
                                  in_=wq_at[kk * 128:(kk + 1) * 128, :])
                    st, sp = kk == 0, kk == KH - 1
                    for mt in range(2):
                        lhs = xch(kk)[:, mt * 128:(mt + 1) * 128]
                        for g in range(3):
                            nc.tensor.matmul(psq[mt][g], lhs,
                                             w_sb[:, g * 512:(g + 1) * 512],
                                             start=st, stop=sp)

                qact = []
                for mt in range(2):
                    actl = p1acc.tile([128, LQ], BF16, tag=f"qact{mt}",
                                      name=f"qact{mt}")
                    qact.append(actl)
                    ps_list = psq[mt]
                    # rms norm q (over 1536)
                    sq = p1sb.tile([128, 512], F32, tag="sq")
                    r3 = p1sb.tile([128, 3], F32, tag="r3")
                    for g in range(3):
                        nc.scalar.activation(sq, ps_list[g], AF.Square,
                                             accum_out=r3[:, g:g + 1])
                    ssum = p1sb.tile([128, 1], F32, tag="ssum")
                    nc.vector.tensor_reduce(ssum, r3, mybir.AxisListType.X,
                                            mybir.AluOpType.add)
                    rtq = p1sb.tile([128, 1], F32, tag="rtq")
                    nc.scalar.activation(rtq, ssum, AF.Sqrt, bias=eps_sb,
                                         scale=1.0 / LQ)
                    rstdq = p1sb.tile([128, 1], F32, tag="rstdq")
                    nc.vector.reciprocal(rstdq, rtq)
                    for g in range(3):
                        nc.vector.tensor_scalar_mul(
                            actl[:, g * 512:(g + 1) * 512], ps_list[g], rstdq)

                p1q_stack.close()

                # transpose q activations -> [LQ, SC] and bounce+gather
                for mt in range(2):
                    for ft in range(KQ):
                        fs = ft * 128
                        tp = p1tps.tile([128, 128], BF16, tag="tp")
                        nc.tensor.transpose(tp, qact[mt][:, fs:fs + 128],
                                            ident)
                        cp = p1sb.tile([128, 128], BF16, tag="cp")
                        nc.scalar.copy(cp, tp)
                        nc.sync.dma_start(
                            out=bounce_act[fs:fs + 128, mt * 128:(mt + 1) * 128],
                            in_=cp)
                allgather(bounce_act, gath_act)

            # ---------------- Phase 2: up-projections (4 local heads) ----
            from contextlib import ExitStack
            with tc.tile_pool(name="attn", bufs=1) as attn_pool, \
                 tc.tile_pool(name="p34w", bufs=1) as p34w:
              with ExitStack() as p2stack:
                p2acts = p2stack.enter_context(tc.tile_pool(name="p2acts", bufs=1))
                p2w = p2stack.enter_context(tc.tile_pool(name="p2w", bufs=1))
                p2ps = p2stack.enter_context(tc.tile_pool(name="p2ps", bufs=3, space="PSUM"))
                p2tps = p2stack.enter_context(tc.tile_pool(name="p2tps", bufs=2, space="PSUM"))
                p2sb = p2stack.enter_context(tc.tile_pool(name="p2sb", bufs=3))

                actkvT = []
                for ft in range(NFKV):
                    fw = min(128, KVD - ft * 128)
                    a = p2acts.tile([128, NCORE, SC], BF16, tag=f"akv{ft}")
                    nc.sync.dma_start(
                        out=a[:fw],
                        in_=gath_act[:, LQ + ft * 128:LQ + ft * 128 + fw, :]
                            .rearrange("r p s -> p r s"))
                    actkvT.append(a.rearrange("p r s -> p (r s)"))

                wkvbn_sb = p2w.tile([128, KKV, HL * DN], BF16, tag="wkvbn")
                nc.scalar.dma_start(out=wkvbn_sb,
                                    in_=wkvbn_t.rearrange("(kk p) n -> p kk n", p=128))
                wkvbv_sb = p2w.tile([128, KKV, HL * DV], BF16, tag="wkvbv")
                nc.scalar.dma_start(out=wkvbv_sb,
                                    in_=wkvbv_t.rearrange("(kk p) n -> p kk n", p=128))
                wqbn_sb = p2w.tile([128, KQ, HL * DN], BF16, tag="wqbn")
                nc.scalar.dma_start(out=wqbn_sb,
                                    in_=wqbn_t.rearrange("(kk p) n -> p kk n", p=128))
                wqbr_sb = p2w.tile([128, KQ, HL * DR], BF16, tag="wqbr")
                nc.scalar.dma_start(out=wqbr_sb,
                                    in_=wqbr_t.rearrange("(kk p) n -> p kk n", p=128))
                cosr_sb = p2w.tile([128, NT, HL * DR // 2], BF16, tag="cosr")
                nc.scalar.dma_start(out=cosr_sb,
                                    in_=cos_r.rearrange("(mt p) i -> p mt i", p=128))
                sinr_sb = p2w.tile([128, NT, HL * DR // 2], BF16, tag="sinr")
                nc.scalar.dma_start(out=sinr_sb,
                                    in_=sin_r.rearrange("(mt p) i -> p mt i", p=128))

                actqT = []
                for ft in range(KQ):
                    a = p2acts.tile([128, NCORE, SC], BF16, tag=f"aq{ft}")
                    nc.sync.dma_start(
                        out=a,
                        in_=gath_act[:, ft * 128:(ft + 1) * 128, :]
                            .rearrange("r p s -> p r s"))
                    actqT.append(a.rearrange("p r s -> p (r s)"))

                # shared rope key, kept for phase 3 after act pool closes
                kpe_sb = attn_pool.tile([64, S], BF16, tag="kpe")
                nc.vector.tensor_copy(kpe_sb, actkvT[NFKV - 1][:64, :])

                # k_nope.T  [128, S] per head  (kv side first: gather ready)
                knT = [attn_pool.tile([128, S], BF16, tag=f"knT{h}", name=f"knT{h}")
                       for h in range(HL)]
                for h in range(HL):
                    for g in range(NG):
                        ps = p2ps.tile([128, 512], F32, tag="ps2")
                        for kk in range(KKV):
                            nc.tensor.matmul(ps, wkvbn_sb[:, kk, h * 128:(h + 1) * 128],
                                             actkvT[kk][:, g * 512:(g + 1) * 512],
                                             start=kk == 0, stop=kk == KKV - 1)
                        nc.scalar.copy(knT[h][:, g * 512:(g + 1) * 512], ps)
                # v natural [S, HL*DV] as 16 tiles [128, 512]
                v_sb = [attn_pool.tile([128, HL * DV], BF16, tag=f"v{mt}", name=f"v{mt}")
                        for mt in range(NT)]
                for mt in range(NT):
                    ps = p2ps.tile([128, 512], F32, tag="ps2")
                    for kk in range(KKV):
                        nc.tensor.matmul(ps, actkvT[kk][:, mt * 128:(mt + 1) * 128],
                                         wkvbv_sb[:, kk, :],
                                         start=kk == 0, stop=kk == KKV - 1)
                    nc.scalar.copy(v_sb[mt], ps)
                # q_nope.T  [128, S] per head
                qnT = [attn_pool.tile([128, S], BF16, tag=f"qnT{h}", name=f"qnT{h}")
                       for h in range(HL)]
                for h in range(HL):
                    for g in range(NG):
                        ps = p2ps.tile([128, 512], F32, tag="ps2")
                        for kk in range(KQ):
                            nc.tensor.matmul(ps, wqbn_sb[:, kk, h * 128:(h + 1) * 128],
                                             actqT[kk][:, g * 512:(g + 1) * 512],
                                             start=kk == 0, stop=kk == KQ - 1)
                        nc.scalar.copy(qnT[h][:, g * 512:(g + 1) * 512], ps)
                # q_pe natural, rope, then transpose into [64, S] tiles
                qpeT = [attn_pool.tile([64, S], BF16, tag=f"qpeT{i}", name=f"qpeT{i}")
                        for i in range(HL)]
                for mt in range(NT):
                    ps = p2ps.tile([128, HL * DR], F32, tag="psqpe")
                    for kk in range(KQ):
                        nc.tensor.matmul(ps, actqT[kk][:, mt * 128:(mt + 1) * 128],
                                         wqbr_sb[:, kk, :],
                                         start=kk == 0, stop=kk == KQ - 1)
                    pv = ps.rearrange("p (h i two) -> p h i two", h=HL, two=2)
                    x1, x2 = pv[:, :, :, 0], pv[:, :, :, 1]
                    cs = cosr_sb[:, mt, :].rearrange("p (h i) -> p h i", h=HL)
                    sn = sinr_sb[:, mt, :].rearrange("p (h i) -> p h i", h=HL)
                    qp = p2sb.tile([128, HL * DR], BF16, tag="qp")
                    qpv = qp.rearrange("p (h i two) -> p h i two", h=HL, two=2)
                    t1 = p2sb.tile([128, HL * DR // 2], F32, tag="t1")
                    t1v = t1.rearrange("p (h i) -> p h i", h=HL)
                    t2 = p2sb.tile([128, HL * DR // 2], F32, tag="t2")
                    t2v = t2.rearrange("p (h i) -> p h i", h=HL)
                    nc.vector.tensor_mul(t1v, x1, cs)
                    nc.vector.tensor_mul(t2v, x2, sn)
                    nc.vector.tensor_sub(qpv[:, :, :, 0], t1v, t2v)
                    nc.vector.tensor_mul(t1v, x1, sn)
                    nc.vector.tensor_mul(t2v, x2, cs)
                    nc.vector.tensor_add(qpv[:, :, :, 1], t1v, t2v)
                    for h in range(HL):
                        tp = p2tps.tile([64, 128], BF16, tag="tpq")
                        nc.tensor.transpose(tp, qp[:, h * DR:(h + 1) * DR], ident)
                        nc.scalar.copy(
                            qpeT[h][:, mt * 128:(mt + 1) * 128], tp)

              # wo slice weights: load during attention, used by phase 4
              wo_sb = p34w.tile([128, KH, 512], BF16, tag="wo")
              nc.sync.dma_start(out=wo_sb,
                                in_=wo_t.rearrange("(kk p) n -> p kk n", p=128))

              # ------- Phase 3+4: causal attention pipelined with wo ----
              with tc.tile_pool(name="p3ps_s", bufs=3, space="PSUM") as p3s, \
                   tc.tile_pool(name="p3ps_o", bufs=2, space="PSUM") as p3o, \
                   tc.tile_pool(name="p3ps_d", bufs=1, space="PSUM") as p3d, \
                   tc.tile_pool(name="p4ps", bufs=2, space="PSUM") as p4ps, \
                   tc.tile_pool(name="p3sb", bufs=2) as p3sb, \
                   tc.tile_pool(name="p4a", bufs=4) as p4a, \
                   tc.tile_pool(name="p4sb", bufs=2) as p4sb:

                  for g in [3, 2, 1, 0]:
                      nk = 4 * g + 4
                      for h in range(HL):
                          qn, kn = qnT[h], knT[h]
                          qp = qpeT[h]
                          ps_o = p3o.tile([128, 512], F32, tag="ps_o")
                          acc = p3sb.tile([128, 512], F32, tag="acc")
                          for c in range(nk):
                              sdiag = c - 4 * g
                              off = 128 * sdiag if sdiag >= 0 else 0
                              ps_s = p3s.tile([128, 512], F32, tag="ps_s")
                              # columns < off are fully causal-masked: skip
                              nc.tensor.matmul(
                                  ps_s[:, off:],
                                  kn[:, c * 128:(c + 1) * 128],
                                  qn[:, g * 512 + off:(g + 1) * 512],
                                  start=True, stop=False)
                              nc.tensor.matmul(
                                  ps_s[:, off:],
                                  kpe_sb[:, c * 128:(c + 1) * 128],
                                  qp[:, g * 512 + off:(g + 1) * 512],
                                  start=False, stop=True)
                              if sdiag >= 0:
                                  nc.vector.tensor_add(
                                      ps_s[:, off:off + 128],
                                      ps_s[:, off:off + 128], dmask_sb)
                              pt = p3sb.tile([128, 512], BF16, tag="pt", bufs=4)
                              nc.scalar.activation(pt[:, off:], ps_s[:, off:],
                                                   AF.Exp)
                              nc.tensor.matmul(
                                  ps_o[:, off:],
                                  v_sb[c][:, h * 128:(h + 1) * 128],
                                  pt[:, off:],
                                  start=c == 0, stop=c == nk - 1)
                              if c == 0:
                                  nc.vector.tensor_copy(acc, pt)
                              else:
                                  nc.vector.tensor_add(acc[:, off:],
                                                       acc[:, off:],
                                                       pt[:, off:])
                          accb = p3sb.tile([128, 512], BF16, tag="accb")
                          nc.vector.tensor_copy(accb, acc)
                          psd = p3d.tile([1, 512], F32, tag="psd")
                          nc.tensor.matmul(psd, ones_k, accb,
                                           start=True, stop=True)
                          rec = p3sb.tile([1, 512], F32, tag="rec")
                          nc.vector.reciprocal(rec, psd)
                          rb = p3sb.tile([128, 512], F32, tag="rb")
                          nc.gpsimd.partition_broadcast(rb, rec)
                          outg = p3sb.tile([128, 512], BF16, tag="outg",
                                           bufs=3)
                          nc.vector.tensor_mul(outg, ps_o, rb)
                          nc.sync.dma_start(
                              out=bounce_o[h * 128:(h + 1) * 128,
                                           g * 512:(g + 1) * 512],
                              in_=outg)
                  allgather(bounce_o, gath_o)
                  # ---- Phase 4: output projection slice ----
                  for mt in range(NT):
                      a_sb = p4a.tile([128, KH, 128], BF16, tag="a")
                      nc.gpsimd.dma_start(
                          out=a_sb,
                          in_=gath_o[:, :, mt * 128:(mt + 1) * 128]
                              .rearrange("r (q4 p) s -> p r q4 s", p=128)
                              .rearrange("p r q4 s -> p (r q4) s"))
                      ps = p4ps.tile([128, 512], F32, tag="psf")
                      for kk in range(KH):
                          nc.tensor.matmul(ps, a_sb[:, kk, :],
                                           wo_sb[:, kk, :],
                                           start=kk == 0, stop=kk == KH - 1)
                      f_sb = p4sb.tile([128, 512], F32, tag="f")
                      nc.vector.tensor_copy(f_sb, ps)
                      nc.sync.dma_start(out=out_c[mt * 128:(mt + 1) * 128, :],
                                        in_=f_sb)

    nc.compile()
    return nc


def _prep(x, wq_a, q_norm_w, wq_b, wkv_a, kv_norm_w, wkv_b, wo):
    bf = ml_dtypes.bfloat16
    f32 = np.float32
    x2 = np.asarray(x, f32).reshape(S, H)
    xT = np.ascontiguousarray(x2.T).astype(bf)                    # [H, S]
    wq_aT = np.ascontiguousarray(np.asarray(wq_a, f32).T).astype(bf)
    wkv_aT = np.ascontiguousarray(np.asarray(wkv_a, f32).T).astype(bf)

    scale = 1.0 / np.sqrt(np.float32(DN + DR))
    wq_b_eff = (np.asarray(wq_b, f32) * np.asarray(q_norm_w, f32)[None, :]
                * scale).reshape(NH, DN + DR, LQ)
    wkv_b_eff = (np.asarray(wkv_b, f32)
                 * np.asarray(kv_norm_w, f32)[None, :]).reshape(NH, DN + DV, LKV)

    # rope tables (mirror reference fp32 math)
    freqs = 1.0 / (10000.0 ** (np.arange(0, DR, 2, dtype=f32) / DR))
    t = np.arange(S, dtype=f32)
    ang = np.outer(t, freqs)                                      # [S, 32]
    cos, sin = np.cos(ang).astype(f32), np.sin(ang).astype(f32)
    cos_rep = np.tile(cos, (1, HL)).astype(bf)                    # [S, 128]
    sin_rep = np.tile(sin, (1, HL)).astype(bf)

    dm = np.where(np.arange(128)[:, None] > np.arange(128)[None, :],
                  np.float32(NEG), np.float32(0.0))

    in_maps = []
    for c in range(NCORE):
        hs = slice(c * HL, (c + 1) * HL)
        wqbn = wq_b_eff[hs, :DN, :].reshape(HL * DN, LQ)
        wqbr = wq_b_eff[hs, DN:, :].reshape(HL * DR, LQ)
        wkvbn = wkv_b_eff[hs, :DN, :].reshape(HL * DN, LKV)
        wkvbv = wkv_b_eff[hs, DN:, :].reshape(HL * DV, LKV)
        in_maps.append({
            "x_t": np.ascontiguousarray(xT[:, c * SC:(c + 1) * SC]),
            "wq_at": wq_aT,
            "wkv_at": wkv_aT,
            "wqbn_t": np.ascontiguousarray(wqbn.T).astype(bf),
            "wqbr_t": np.ascontiguousarray(wqbr.T).astype(bf),
            "wkvbn_t": np.ascontiguousarray(wkvbn.T).astype(bf),
            "wkvbv_t": np.ascontiguousarray(wkvbv.T).astype(bf),
            "wo_t": np.ascontiguousarray(
                np.asarray(wo, f32)[c * 512:(c + 1) * 512, :].T).astype(bf),
            "cos_k": np.ascontiguousarray(cos[c * SC:(c + 1) * SC]),
            "sin_k": np.ascontiguousarray(sin[c * SC:(c + 1) * SC]),
            "cos_r": cos_rep,
            "sin_r": sin_rep,
            "dmask": dm,
        })
    return in_maps


def kernel(x, wq_a, q_norm_w, wq_b, wkv_a, kv_norm_w, wkv_b, wo,
           _trace=False):
    if "nc" not in _compiled:
        _compiled["nc"] = _build()
    nc = _compiled["nc"]
    in_maps = _prep(x, wq_a, q_norm_w, wq_b, wkv_a, kv_norm_w, wkv_b, wo)
    try:
        res = run_bass_kernel_spmd(nc, in_maps, list(range(NCORE)),
                                   trace=_trace)
    except Exception:
        # transient NRT/device wedge: one retry after a short pause
        import time as _time
        _time.sleep(15)
        res = run_bass_kernel_spmd(nc, in_maps, list(range(NCORE)),
                                   trace=_trace)
    _compiled["last_result"] = res
    out = np.concatenate([res.results[c]["out_c"] for c in range(NCORE)],
                         axis=1)
    return out.reshape(B, S, NH * DV).astype(np.float32)

